# revision 18
# baseline (speedup 1.0000x reference)
"""Trainium2 Bass kernel for nn_Attention3D (B=4, C=256, D=H=W=16).

y = x + wp @ softmax_j((wq@x+bq)^T (wk@x+bk) / sqrt(C)) applied to (wv@x+bv), + bp

Sharding: 8 cores = (batch b, query-half). Each core owns one batch's full
K/V (N=4096 keys) and half the queries (2048). Key order is permuted per
core so "my" queries are always columns 0:2048 — softmax/attention are
invariant to key permutation, so every core runs the identical program.

Per-core pipeline:
  projections (bf16 matmul, fp32 PSUM):   q,k (c-major), vT (j-major, built
    directly transposed so no on-chip transposes are ever needed)
  attention (fp8 e4m3, DoubleRow: virtual K=256, 2 MACs/cycle):
    sT[j,i] = k^T q       exp on ScalarE (scale 1/16, output scaled 2^-4
    so e fits e4m3; the scale cancels in the softmax ratio)
    attn_un = vT^T e      S = ones^T e   (softmax denominator as a matmul,
    accumulated in PSUM; normalization deferred past the out-projection)
  epilogue: y = (wp^T attn_un) * (1/S) + bp + x   in fp32
The k/vT projections for the later key blocks are emitted lazily inside the
first i-chunk's superblock loop, so compute overlaps the x16 input DMA.
"""

import numpy as np
import ml_dtypes

B, C = 4, 256
D = H = W = 16
N = D * H * W          # 4096 voxels
P = 128                # partitions
CB = C // P            # 2 channel blocks
NI = N // 2            # 2048 queries per core
NCORES = 8
IC = 512               # i-chunk (one PSUM bank of fp32)
NIC = NI // IC         # 4 i-chunks
NJ = N // P            # 32 key blocks
NJ2 = NJ // 2          # 16 key superblocks (256 keys each, fp8 DoubleRow)
NKC = N // IC          # 8 key projection chunks
ESHIFT = -4 * 0.6931471805599453  # exp bias: fold 2^-4 so e fits fp8 e4m3
LA = 3                 # superblock lookahead for lazy projection emission

_cache = {}


def _build():
    import concourse.bacc as bacc
    import concourse.mybir as mybir
    import concourse.tile as tile

    dt = mybir.dt
    f32, bf16, f8 = dt.float32, dt.bfloat16, dt.float8e4

    nc = bacc.Bacc("TRN2", target_bir_lowering=False, debug=False)

    x16_d = nc.dram_tensor("x16", [C, N], bf16, kind="ExternalInput")
    xq_d = nc.dram_tensor("xq32", [C, NI], f32, kind="ExternalInput")
    w_d = {
        w: nc.dram_tensor(w, [C, C], bf16, kind="ExternalInput")
        for w in ("wqT", "wkT", "wvT", "wpT")
    }
    bq_d = nc.dram_tensor("bq", [C, 1], f32, kind="ExternalInput")
    bk_d = nc.dram_tensor("bk", [C, 1], f32, kind="ExternalInput")
    bv_d = nc.dram_tensor("bv", [1, C], f32, kind="ExternalInput")
    bp_d = nc.dram_tensor("bp", [C, 1], f32, kind="ExternalInput")
    y_d = nc.dram_tensor("y", [C, NI], f32, kind="ExternalOutput")

    add = mybir.AluOpType.add
    EXP = mybir.ActivationFunctionType.Exp
    DR = mybir.MatmulPerfMode.DoubleRow

    with tile.TileContext(nc) as tc:
        with (
            tc.tile_pool(name="consts", bufs=1) as consts,
            tc.tile_pool(name="acts", bufs=1) as acts,
            tc.tile_pool(name="e16p", bufs=14) as e16p,
            tc.tile_pool(name="small", bufs=2) as small,
            tc.tile_pool(name="ys", bufs=3) as ys,
            tc.tile_pool(name="ps_s", bufs=2, space="PSUM") as ps_s,
            tc.tile_pool(name="ps_a", bufs=3, space="PSUM") as ps_a,
            tc.tile_pool(name="ps_S", bufs=1, space="PSUM") as ps_S,
        ):
            # ---- constants ----
            w_sb = {}
            for wname in ("wqT", "wkT", "wvT", "wpT"):
                w_sb[wname] = []
                for cb in range(CB):
                    t = consts.tile([P, C], bf16, tag=f"{wname}{cb}", name=f"{wname}{cb}")
                    nc.gpsimd.dma_start(out=t, in_=w_d[wname].ap()[cb * P:(cb + 1) * P, :])
                    w_sb[wname].append(t)
            bias_sb = {}
            for bname, bd in (("bq", bq_d), ("bk", bk_d), ("bp", bp_d)):
                bias_sb[bname] = []
                for cb in range(CB):
                    t = consts.tile([P, 1], f32, tag=f"{bname}{cb}", name=f"{bname}{cb}")
                    nc.gpsimd.dma_start(out=t, in_=bd.ap()[cb * P:(cb + 1) * P, :])
                    bias_sb[bname].append(t)
            bv_b = consts.tile([P, C], f32, tag="bv_b")
            nc.gpsimd.dma_start(out=bv_b, in_=bv_d.ap().to_broadcast([P, C]))
            ones8_t = consts.tile([P, 2, P], f8, tag="ones8")
            nc.vector.memset(ones8_t, 1.0)
            eshift_t = consts.tile([P, 1], f32, tag="eshift")
            nc.vector.memset(eshift_t, ESHIFT)

            # ---- input x (bf16), split DMAs so queues parallelize;
            #      query half (cols 0:2048) of both c-blocks first ----
            x16 = [acts.tile([P, N], bf16, tag=f"x16_{cb}", name=f"x16_{cb}")
                   for cb in range(CB)]
            for ch in range(4):
                for cb in range(CB):
                    csl = slice(ch * (N // 4), (ch + 1) * (N // 4))
                    eng = nc.sync if cb == 0 else nc.gpsimd
                    eng.dma_start(out=x16[cb][:, csl],
                                  in_=x16_d.ap()[cb * P:(cb + 1) * P, csl])

            # residual input, streamed per i-chunk during phase 2
            xq32 = [acts.tile([P, NI], f32, tag=f"xq32_{cb}", name=f"xq32_{cb}")
                    for cb in range(CB)]

            # activations: fp8 pair-interleaved (channel c = pair*128 + ci)
            q16 = acts.tile([P, CB, NI], f8, tag="q16")    # [ci, pair, i]
            k16 = acts.tile([P, CB, N], f8, tag="k16")     # [ci, pair, j]
            vT16 = acts.tile([P, NJ2, 2, C], f8, tag="vT16")  # [ji, sb, pair, c]

            # ---- projection emitters (drained lazily against the DMA) ----
            def emit_qproj(ic):
                isl = slice(ic * IC, (ic + 1) * IC)
                for ob in range(CB):
                    ps = ps_s.tile([P, IC], f32, tag="ps_s", name="qps")
                    for cb in range(CB):
                        nc.tensor.matmul(
                            ps, w_sb["wqT"][cb][:, ob * P:(ob + 1) * P],
                            x16[cb][:, isl],
                            start=(cb == 0), stop=(cb == CB - 1))
                    nc.vector.tensor_scalar_add(q16[:, ob, isl], ps, bias_sb["bq"][ob])

            def emit_kproj(jc):
                jsl = slice(jc * IC, (jc + 1) * IC)
                for ob in range(CB):
                    ps = ps_a.tile([P, IC], f32, tag="ps_a", name="kps")
                    for cb in range(CB):
                        nc.tensor.matmul(
                            ps, w_sb["wkT"][cb][:, ob * P:(ob + 1) * P],
                            x16[cb][:, jsl],
                            start=(cb == 0), stop=(cb == CB - 1))
                    nc.vector.tensor_scalar_add(k16[:, ob, jsl], ps, bias_sb["bk"][ob])

            def emit_vproj(j):
                ps = ps_a.tile([P, C], f32, tag="ps_a", name="vps")
                for cb in range(CB):
                    nc.tensor.matmul(
                        ps, x16[cb][:, j * P:(j + 1) * P], w_sb["wvT"][cb],
                        start=(cb == 0), stop=(cb == CB - 1))
                nc.vector.tensor_add(vT16[:, j // 2, j % 2, :], ps, bv_b)

            # interleave projections so early superblocks unlock promptly
            emit_qproj(0)
            for g in range(NKC):
                if g and g % 2 == 0:
                    emit_qproj(g // 2)
                emit_kproj(g)
                for j in range(4 * g, 4 * g + 4):
                    emit_vproj(j)

            # ---- attention, one i-chunk (512 queries) at a time ----
            for ic in range(NIC):
                isl = slice(ic * IC, (ic + 1) * IC)
                for cb in range(CB):
                    nc.gpsimd.dma_start(out=xq32[cb][:, isl],
                                        in_=xq_d.ap()[cb * P:(cb + 1) * P, isl])
                a_ps = [ps_a.tile([P, IC], f32, tag="ps_a", name=f"a_ps{cb}")
                        for cb in range(CB)]
                S_ps = ps_S.tile([P, IC], f32, tag="ps_S")
                for sb in range(NJ2):
                    s_ps = ps_s.tile([P, 2 * IC], f32, tag="ps_s")
                    for r in range(2):
                        jb = 2 * sb + r
                        nc.tensor.matmul(
                            s_ps[:, r * IC:(r + 1) * IC],
                            k16[:, :, jb * P:(jb + 1) * P], q16[:, :, isl],
                            start=True, stop=True, perf_mode=DR)
                    e16 = e16p.tile([P, 2, IC], f8, tag="e16")
                    nc.scalar.activation(e16, s_ps, EXP,
                                         scale=float(C) ** -0.5, bias=eshift_t)
                    first, last = (sb == 0), (sb == NJ2 - 1)
                    for cb in range(CB):
                        nc.tensor.matmul(
                            a_ps[cb], vT16[:, sb, :, cb * P:(cb + 1) * P], e16,
                            start=first, stop=last, perf_mode=DR)
                    if sb > 0:
                        nc.tensor.matmul(S_ps, ones8_t, e16_prev,
                                         start=(sb == 1), stop=False, perf_mode=DR)
                    e16_prev = e16
                nc.tensor.matmul(S_ps, ones8_t, e16_prev,
                                 start=False, stop=True, perf_mode=DR)
                R = small.tile([P, IC], f32, tag="R")
                nc.vector.reciprocal_approx_fast(out=R, in_=S_ps)
                attn16 = [small.tile([P, IC], bf16, tag=f"at{cb}", name=f"at{cb}")
                          for cb in range(CB)]
                for cb in range(CB):
                    nc.vector.tensor_copy(attn16[cb], a_ps[cb])
                for ob in range(CB):
                    o_ps = ps_a.tile([P, IC], f32, tag="ps_a", name="o_ps")
                    for cb in range(CB):
                        nc.tensor.matmul(
                            o_ps, w_sb["wpT"][cb][:, ob * P:(ob + 1) * P], attn16[cb],
                            start=(cb == 0), stop=(cb == CB - 1))
                    tmp = ys.tile([P, IC], f32, tag="tmp")
                    nc.vector.tensor_mul(tmp, o_ps, R)
                    yt = ys.tile([P, IC], f32, tag="yt")
                    nc.vector.scalar_tensor_tensor(
                        yt, tmp, bias_sb["bp"][ob], xq32[ob][:, isl],
                        op0=add, op1=add)
                    nc.sync.dma_start(out=y_d.ap()[ob * P:(ob + 1) * P, isl], in_=yt)

    nc.compile()
    return nc


def _prep_inputs(x, wq, bq, wk, bk, wv, bv, wp, bp):
    bf16 = ml_dtypes.bfloat16
    xf = np.asarray(x, np.float32).reshape(B, C, N)
    shared = {
        "wqT": np.ascontiguousarray(np.asarray(wq, np.float32).T).astype(bf16),
        "wkT": np.ascontiguousarray(np.asarray(wk, np.float32).T).astype(bf16),
        "wvT": np.ascontiguousarray(np.asarray(wv, np.float32).T).astype(bf16),
        "wpT": np.ascontiguousarray(np.asarray(wp, np.float32).T).astype(bf16),
        "bq": np.asarray(bq, np.float32).reshape(C, 1),
        "bk": np.asarray(bk, np.float32).reshape(C, 1),
        "bv": np.asarray(bv, np.float32).reshape(1, C),
        "bp": np.asarray(bp, np.float32).reshape(C, 1),
    }
    in_maps = []
    for core in range(NCORES):
        b, h = core // 2, core % 2
        xs = xf[b]
        if h == 1:  # roll so this core's query half is first (key order irrelevant)
            xs = np.concatenate([xs[:, NI:], xs[:, :NI]], axis=1)
        m = dict(shared)
        m["x16"] = np.ascontiguousarray(xs).astype(bf16)
        m["xq32"] = np.ascontiguousarray(xs[:, :NI], np.float32)
        in_maps.append(m)
    return in_maps


def _run(inputs, trace=False, **kwargs):
    from concourse.bass_utils import run_bass_kernel_spmd

    if "nc" not in _cache:
        _cache["nc"] = _build()
    nc = _cache["nc"]
    in_maps = _prep_inputs(**inputs)
    res = run_bass_kernel_spmd(
        nc, in_maps, core_ids=list(range(NCORES)), trace=trace, **kwargs
    )
    out = np.empty((B, C, N), np.float32)
    for core in range(NCORES):
        b, h = core // 2, core % 2
        out[b][:, h * NI:(h + 1) * NI] = res.results[core]["y"]
    return out.reshape(B, C, D, H, W), res


def kernel(**inputs):
    out, _ = _run(inputs)
    return out


# revision 19
# speedup vs baseline: 1.0513x; 1.0513x over previous
"""Trainium2 Bass kernel for nn_Attention3D (B=4, C=256, D=H=W=16).

y = x + wp @ softmax_j((wq@x+bq)^T (wk@x+bk) / sqrt(C)) applied to (wv@x+bv), + bp

Sharding: 8 cores = (batch b, query-half). Each core owns one batch's full
K/V (N=4096 keys) and half the queries (2048). Key order is permuted per
core so "my" queries are always columns 0:2048 — softmax/attention are
invariant to key permutation, so every core runs the identical program.

Per-core pipeline:
  projections (bf16 matmul, fp32 PSUM):   q,k (c-major), vT (j-major, built
    directly transposed so no on-chip transposes are ever needed)
  attention (fp8 e4m3, DoubleRow: virtual K=256, 2 MACs/cycle):
    sT[j,i] = k^T q       exp on ScalarE (scale 1/16, output scaled 2^-4
    so e fits e4m3; the scale cancels in the softmax ratio)
    attn_un = vT^T e      S = ones^T e   (softmax denominator as a matmul,
    accumulated in PSUM; normalization deferred past the out-projection)
  epilogue: y = (wp^T attn_un) * (1/S) + bp + x   in fp32
The k/vT projections for the later key blocks are emitted lazily inside the
first i-chunk's superblock loop, so compute overlaps the x16 input DMA.
"""

import numpy as np
import ml_dtypes

B, C = 4, 256
D = H = W = 16
N = D * H * W          # 4096 voxels
P = 128                # partitions
CB = C // P            # 2 channel blocks
NI = N // 2            # 2048 queries per core
NCORES = 8
IC = 512               # i-chunk (one PSUM bank of fp32)
NIC = NI // IC         # 4 i-chunks
NJ = N // P            # 32 key blocks
NJ2 = NJ // 2          # 16 key superblocks (256 keys each, fp8 DoubleRow)
NKC = N // IC          # 8 key projection chunks
ESHIFT = -4 * 0.6931471805599453  # exp bias: fold 2^-4 so e fits fp8 e4m3
LA = 3                 # superblock lookahead for lazy projection emission

_cache = {}


def _build():
    import concourse.bacc as bacc
    import concourse.mybir as mybir
    import concourse.tile as tile

    dt = mybir.dt
    f32, bf16, f8 = dt.float32, dt.bfloat16, dt.float8e4

    nc = bacc.Bacc("TRN2", target_bir_lowering=False, debug=False)

    x16_d = nc.dram_tensor("x16", [C, N], bf16, kind="ExternalInput")
    xq_d = nc.dram_tensor("xq32", [C, NI], f32, kind="ExternalInput")
    w_d = {
        w: nc.dram_tensor(w, [C, C], bf16, kind="ExternalInput")
        for w in ("wqT", "wkT", "wvT", "wpT")
    }
    bq_d = nc.dram_tensor("bq", [C, 1], f32, kind="ExternalInput")
    bk_d = nc.dram_tensor("bk", [C, 1], f32, kind="ExternalInput")
    bv_d = nc.dram_tensor("bv", [1, C], f32, kind="ExternalInput")
    bp_d = nc.dram_tensor("bp", [C, 1], f32, kind="ExternalInput")
    y_d = nc.dram_tensor("y", [C, NI], f32, kind="ExternalOutput")

    add = mybir.AluOpType.add
    EXP = mybir.ActivationFunctionType.Exp
    DR = mybir.MatmulPerfMode.DoubleRow

    with tile.TileContext(nc) as tc:
        with (
            tc.tile_pool(name="consts", bufs=1) as consts,
            tc.tile_pool(name="acts", bufs=1) as acts,
            tc.tile_pool(name="e16p", bufs=14) as e16p,
            tc.tile_pool(name="small", bufs=2) as small,
            tc.tile_pool(name="ys", bufs=3) as ys,
            tc.tile_pool(name="ps_s", bufs=2, space="PSUM") as ps_s,
            tc.tile_pool(name="ps_a", bufs=3, space="PSUM") as ps_a,
            tc.tile_pool(name="ps_S", bufs=1, space="PSUM") as ps_S,
        ):
            # ---- constants ----
            w_sb = {}
            for wname in ("wqT", "wkT", "wvT", "wpT"):
                w_sb[wname] = []
                for cb in range(CB):
                    t = consts.tile([P, C], bf16, tag=f"{wname}{cb}", name=f"{wname}{cb}")
                    nc.gpsimd.dma_start(out=t, in_=w_d[wname].ap()[cb * P:(cb + 1) * P, :])
                    w_sb[wname].append(t)
            bias_sb = {}
            for bname, bd in (("bq", bq_d), ("bk", bk_d), ("bp", bp_d)):
                bias_sb[bname] = []
                for cb in range(CB):
                    t = consts.tile([P, 1], f32, tag=f"{bname}{cb}", name=f"{bname}{cb}")
                    nc.gpsimd.dma_start(out=t, in_=bd.ap()[cb * P:(cb + 1) * P, :])
                    bias_sb[bname].append(t)
            bv_b = consts.tile([P, C], f32, tag="bv_b")
            nc.gpsimd.dma_start(out=bv_b, in_=bv_d.ap().to_broadcast([P, C]))
            ones8_t = consts.tile([P, 2, P], f8, tag="ones8")
            nc.vector.memset(ones8_t, 1.0)
            eshift_t = consts.tile([P, 1], f32, tag="eshift")
            nc.vector.memset(eshift_t, ESHIFT)

            # ---- input x (bf16), split DMAs so queues parallelize;
            #      query half (cols 0:2048) of both c-blocks first ----
            x16 = [acts.tile([P, N], bf16, tag=f"x16_{cb}", name=f"x16_{cb}")
                   for cb in range(CB)]
            for ch in range(4):
                for cb in range(CB):
                    csl = slice(ch * (N // 4), (ch + 1) * (N // 4))
                    eng = nc.sync if cb == 0 else nc.gpsimd
                    eng.dma_start(out=x16[cb][:, csl],
                                  in_=x16_d.ap()[cb * P:(cb + 1) * P, csl])

            # residual input, streamed per i-chunk during phase 2
            xq32 = [acts.tile([P, NI], f32, tag=f"xq32_{cb}", name=f"xq32_{cb}")
                    for cb in range(CB)]

            # activations: fp8 pair-interleaved (channel c = pair*128 + ci)
            q16 = acts.tile([P, CB, NI], f8, tag="q16")    # [ci, pair, i]
            k16 = acts.tile([P, CB, N], f8, tag="k16")     # [ci, pair, j]
            vT16 = acts.tile([P, NJ2, 2, C], f8, tag="vT16")  # [ji, sb, pair, c]

            # ---- projection emitters (drained lazily against the DMA) ----
            def emit_qproj(ic):
                isl = slice(ic * IC, (ic + 1) * IC)
                for ob in range(CB):
                    ps = ps_s.tile([P, IC], f32, tag="ps_s", name="qps")
                    for cb in range(CB):
                        nc.tensor.matmul(
                            ps, w_sb["wqT"][cb][:, ob * P:(ob + 1) * P],
                            x16[cb][:, isl],
                            start=(cb == 0), stop=(cb == CB - 1))
                    nc.vector.tensor_scalar_add(q16[:, ob, isl], ps, bias_sb["bq"][ob])

            def emit_kproj(jc):
                jsl = slice(jc * IC, (jc + 1) * IC)
                for ob in range(CB):
                    ps = ps_a.tile([P, IC], f32, tag="ps_a", name="kps")
                    for cb in range(CB):
                        nc.tensor.matmul(
                            ps, w_sb["wkT"][cb][:, ob * P:(ob + 1) * P],
                            x16[cb][:, jsl],
                            start=(cb == 0), stop=(cb == CB - 1))
                    nc.vector.tensor_scalar_add(k16[:, ob, jsl], ps, bias_sb["bk"][ob])

            def emit_vproj(j):
                ps = ps_a.tile([P, C], f32, tag="ps_a", name="vps")
                for cb in range(CB):
                    nc.tensor.matmul(
                        ps, x16[cb][:, j * P:(j + 1) * P], w_sb["wvT"][cb],
                        start=(cb == 0), stop=(cb == CB - 1))
                nc.vector.tensor_add(vT16[:, j // 2, j % 2, :], ps, bv_b)

            for ic in range(NIC):
                emit_qproj(ic)
            for jc in range(NKC):
                emit_kproj(jc)
            for j in range(NJ):
                emit_vproj(j)

            # ---- attention, one i-chunk (512 queries) at a time ----
            for ic in range(NIC):
                isl = slice(ic * IC, (ic + 1) * IC)
                for cb in range(CB):
                    nc.gpsimd.dma_start(out=xq32[cb][:, isl],
                                        in_=xq_d.ap()[cb * P:(cb + 1) * P, isl])
                a_ps = [ps_a.tile([P, IC], f32, tag="ps_a", name=f"a_ps{cb}")
                        for cb in range(CB)]
                S_ps = ps_S.tile([P, IC], f32, tag="ps_S")
                for sb in range(NJ2):
                    s_ps = ps_s.tile([P, 2 * IC], f32, tag="ps_s")
                    for r in range(2):
                        jb = 2 * sb + r
                        nc.tensor.matmul(
                            s_ps[:, r * IC:(r + 1) * IC],
                            k16[:, :, jb * P:(jb + 1) * P], q16[:, :, isl],
                            start=True, stop=True, perf_mode=DR)
                    e16 = e16p.tile([P, 2, IC], f8, tag="e16")
                    nc.scalar.activation(e16, s_ps, EXP,
                                         scale=float(C) ** -0.5, bias=eshift_t)
                    first, last = (sb == 0), (sb == NJ2 - 1)
                    for cb in range(CB):
                        nc.tensor.matmul(
                            a_ps[cb], vT16[:, sb, :, cb * P:(cb + 1) * P], e16,
                            start=first, stop=last, perf_mode=DR)
                    if sb > 0:
                        nc.tensor.matmul(S_ps, ones8_t, e16_prev,
                                         start=(sb == 1), stop=False, perf_mode=DR)
                    e16_prev = e16
                nc.tensor.matmul(S_ps, ones8_t, e16_prev,
                                 start=False, stop=True, perf_mode=DR)
                R = small.tile([P, IC], f32, tag="R")
                nc.vector.reciprocal_approx_fast(out=R, in_=S_ps)
                attn16 = [small.tile([P, IC], bf16, tag=f"at{cb}", name=f"at{cb}")
                          for cb in range(CB)]
                for cb in range(CB):
                    nc.vector.tensor_copy(attn16[cb], a_ps[cb])
                for ob in range(CB):
                    o_ps = ps_a.tile([P, IC], f32, tag="ps_a", name="o_ps")
                    for cb in range(CB):
                        nc.tensor.matmul(
                            o_ps, w_sb["wpT"][cb][:, ob * P:(ob + 1) * P], attn16[cb],
                            start=(cb == 0), stop=(cb == CB - 1))
                    tmp = ys.tile([P, IC], f32, tag="tmp")
                    nc.vector.tensor_mul(tmp, o_ps, R)
                    yt = ys.tile([P, IC], f32, tag="yt")
                    nc.vector.scalar_tensor_tensor(
                        yt, tmp, bias_sb["bp"][ob], xq32[ob][:, isl],
                        op0=add, op1=add)
                    nc.sync.dma_start(out=y_d.ap()[ob * P:(ob + 1) * P, isl], in_=yt)

    nc.compile()
    return nc


def _prep_inputs(x, wq, bq, wk, bk, wv, bv, wp, bp):
    bf16 = ml_dtypes.bfloat16
    xf = np.asarray(x, np.float32).reshape(B, C, N)
    shared = {
        "wqT": np.ascontiguousarray(np.asarray(wq, np.float32).T).astype(bf16),
        "wkT": np.ascontiguousarray(np.asarray(wk, np.float32).T).astype(bf16),
        "wvT": np.ascontiguousarray(np.asarray(wv, np.float32).T).astype(bf16),
        "wpT": np.ascontiguousarray(np.asarray(wp, np.float32).T).astype(bf16),
        "bq": np.asarray(bq, np.float32).reshape(C, 1),
        "bk": np.asarray(bk, np.float32).reshape(C, 1),
        "bv": np.asarray(bv, np.float32).reshape(1, C),
        "bp": np.asarray(bp, np.float32).reshape(C, 1),
    }
    in_maps = []
    for core in range(NCORES):
        b, h = core // 2, core % 2
        xs = xf[b]
        if h == 1:  # roll so this core's query half is first (key order irrelevant)
            xs = np.concatenate([xs[:, NI:], xs[:, :NI]], axis=1)
        m = dict(shared)
        m["x16"] = np.ascontiguousarray(xs).astype(bf16)
        m["xq32"] = np.ascontiguousarray(xs[:, :NI], np.float32)
        in_maps.append(m)
    return in_maps


def _run(inputs, trace=False, **kwargs):
    from concourse.bass_utils import run_bass_kernel_spmd

    if "nc" not in _cache:
        _cache["nc"] = _build()
    nc = _cache["nc"]
    in_maps = _prep_inputs(**inputs)
    res = run_bass_kernel_spmd(
        nc, in_maps, core_ids=list(range(NCORES)), trace=trace, **kwargs
    )
    out = np.empty((B, C, N), np.float32)
    for core in range(NCORES):
        b, h = core // 2, core % 2
        out[b][:, h * NI:(h + 1) * NI] = res.results[core]["y"]
    return out.reshape(B, C, D, H, W), res


def kernel(**inputs):
    out, _ = _run(inputs)
    return out
